# revision 11
# baseline (speedup 1.0000x reference)
"""DeepSeek-MoE layer (shared SwiGLU expert + 8 routed GELU experts, top-2)
as a Bass/Tile kernel for 8 Trainium2 NeuronCores.

Sharding: expert-parallel with top-2 sparse token dispatch (the all-to-all
is done host-side as part of sharding, per the full-IO contract). Core e
holds routed expert e's weights and computes:
  - the shared SwiGLU expert for its 512-token data-parallel slice, and
  - gelu(Xg @ ew1[e]) @ ew2[e] scaled by the gate, for the <=cap tokens
    routed to expert e (gathered + zero-padded to a static capacity).
The host computes router scores/top-2/gates (cheap: 67 MFLOP) to build the
dispatch, then scatter-adds the two gated expert outputs per token into the
shared-expert output. This cuts per-core work from dense 23.6 GFLOP / 94 MB
to ~11.3 GFLOP / ~27 MB.

Matmul operands are bf16 (fp32 PSUM accumulation, same PE rate as f32r,
half the HBM bytes); everything else stays fp32. All inputs are pre-packed
host-side into tile-contiguous layouts so each DMA is a handful of large
contiguous descriptors (the descriptor-write cost on the Sync engine is
~5.3 ns/row otherwise).
"""
import sys
sys.path.insert(0, '/opt/trn_rl_repo')

import numpy as np
import ml_dtypes
import concourse.bass as bass
import concourse.tile as tile
from concourse import mybir, bacc
from concourse.bass_utils import run_bass_kernel_spmd

N_CORES = 8
B, T = 2, 2048
D = 1024          # d_model
HS = 2048         # shared-expert hidden
HR = 1024         # routed-expert hidden
E = 8             # experts
TOP_K = 2
NTOK = (B * T) // N_CORES   # shared-expert tokens per core = 512
NCH = NTOK // 128           # shared token chunks of 128 = 4
KD = D // 128               # k-tiles over D = 8
KS = HS // 128              # k-tiles over HS = 16
KR = HR // 128              # k-tiles over HR = 8

F32 = mybir.dt.float32
F32R = mybir.dt.float32r
BF16 = mybir.dt.bfloat16
AF = mybir.ActivationFunctionType
ALU = mybir.AluOpType
AX = mybir.AxisListType
NPBF16 = ml_dtypes.bfloat16

_CACHE = {}


def _blocks(cap):
    """Split cap into near-equal column blocks <=512, multiples of 128."""
    nb = -(-cap // 512)
    per = -(-(cap // 128) // nb)
    out = []
    left = cap // 128
    for _ in range(nb):
        take = min(per, left)
        out.append(take * 128)
        left -= take
    return [b for b in out if b]


def _build(cap, ncl):
    nchr = cap // 128          # routed token chunks of 128
    nb = -(-ncl // 512)
    per = -(-ncl // nb)
    blks, left = [], ncl       # stage-1 routed col blocks over the real cols
    for _ in range(nb):
        take = min(per, left)
        blks.append(take)
        left -= take

    nc = bacc.Bacc(None, target_bir_lowering=False)
    # all inputs pre-packed host-side into the exact tile layouts below
    xt = nc.dram_tensor("xt", [128, KD, NTOK], BF16, kind="ExternalInput")
    xg = nc.dram_tensor("xg", [128, KD, cap], BF16, kind="ExternalInput")
    sw13 = nc.dram_tensor("sw13", [KS, 128, 2, KD, 128], BF16, kind="ExternalInput")
    sw2 = nc.dram_tensor("sw2", [2, 128, KS, 512], BF16, kind="ExternalInput")
    ew1 = nc.dram_tensor("ew1", [128, KD, HR], BF16, kind="ExternalInput")
    ew2 = nc.dram_tensor("ew2", [128, KR, D], BF16, kind="ExternalInput")
    out_sh = nc.dram_tensor("out_sh", [128, NCH, D], BF16, kind="ExternalOutput")
    out_rt = nc.dram_tensor("out_rt", [128, nchr, D], BF16, kind="ExternalOutput")

    with tile.TileContext(nc) as tc:
        with tc.tile_pool(name="persist", bufs=1) as persist, \
             tc.tile_pool(name="wstream", bufs=4) as wstream, \
             tc.tile_pool(name="wsm", bufs=8) as wsm, \
             tc.tile_pool(name="rpool", bufs=2) as rpool, \
             tc.tile_pool(name="opool", bufs=4) as opool, \
             tc.tile_pool(name="oshp", bufs=1) as oshp, \
             tc.tile_pool(name="small", bufs=1) as small, \
             tc.tile_pool(name="psA", bufs=3, space="PSUM") as psA, \
             tc.tile_pool(name="psY", bufs=4, space="PSUM") as psY:

            # ---- PE warm-up burst: drives HAM to K=8/8 while first DMAs land
            wu = small.tile([128, 128], BF16)
            nc.vector.memset(wu[:, :], 1.0)
            pwu = psY.tile([128, 512], F32, tag="py")
            for i in range(36):
                nc.tensor.matmul(pwu[:, 0:128], wu[:, :], wu[:, :],
                                 start=(i == 0), stop=(i == 35))

            # ---- first shared-weight m-pieces, then x halves; xg later
            sw13v = sw13.rearrange("s p two kd m -> p s two kd m")  # [128,16,2,8,128]
            w13s = [None] * KS
            def issue_w13(h2, span):
                t = wsm.tile([128, span, 2, KD, 128], BF16, tag="w13",
                             name=f"w13m{h2}", bufs=6)
                nc.sync.dma_start(out=t, in_=sw13v[:, h2:h2 + span])
                for j in range(span):
                    w13s[h2 + j] = t[:, j]
            issue_w13(0, 1)
            xq = persist.tile([128, KD, NTOK], BF16)
            nc.sync.dma_start(out=xq[:, 0:4, :], in_=xt[:, 0:4, :])
            nc.sync.dma_start(out=xq[:, 4:8, :], in_=xt[:, 4:8, :])
            issue_w13(1, 1)
            issue_w13(2, 2)

            # ---- shared expert stage 1: P = silu(x@sw1) * (x@sw3), f-major
            pshr = persist.tile([128, KS, NTOK], BF16)   # P^T [2048, 512]
            xgq = persist.tile([128, KD, cap], BF16)
            for h2 in range(KS):
                if h2 % 2 == 0 and 4 + h2 < KS:
                    issue_w13(4 + h2, 2)
                if h2 == 4:
                    # routed tokens: not needed until routed stage 1
                    nc.sync.dma_start(out=xgq[:, :, :], in_=xg[:, :, :])
                pa = psA.tile([128, NTOK], F32, tag="pa")
                for k in range(KD):
                    nc.tensor.matmul(pa[:, :], w13s[h2][:, 0, k, :],
                                     xq[:, k, :], start=(k == 0), stop=(k == KD - 1))
                pg = psA.tile([128, NTOK], F32, tag="pa")
                for k in range(KD):
                    nc.tensor.matmul(pg[:, :], w13s[h2][:, 1, k, :],
                                     xq[:, k, :], start=(k == 0), stop=(k == KD - 1))
                asb = rpool.tile([128, NTOK], F32, tag="asb")
                nc.scalar.activation(asb[:, :], pa[:, :], AF.Silu)
                nc.vector.tensor_mul(pshr[:, h2, :], asb[:, :], pg[:, :])

            # ---- shared expert stage 2: out_sh = P @ sw2, token-major
            osh_sb = oshp.tile([128, NCH, D], BF16, tag="osh")
            for dh in range(2):
                pys = [psY.tile([128, 512], F32, tag="py", name=f"py_sh{dh}{c}")
                       for c in range(NCH)]
                w2p = wstream.tile([128, KS, 512], BF16, tag="w",
                                   name=f"w2p{dh}")
                nc.sync.dma_start(out=w2p, in_=sw2[dh])
                for kh in range(2):                       # kt halves of HS
                    for c in range(NCH):
                        for k in range(KD):
                            kk = kh * 8 + k
                            nc.tensor.matmul(
                                pys[c][:, :],
                                pshr[:, kk, c * 128:(c + 1) * 128],
                                w2p[:, kk, :],
                                start=(kk == 0), stop=(kk == KS - 1))
                for c in range(NCH):
                    nc.vector.tensor_copy(osh_sb[:, c, dh * 512:(dh + 1) * 512],
                                          pys[c][:, :])
            nc.sync.dma_start(out=out_sh[:, :, :], in_=osh_sb[:, :, :])

            # ---- routed expert stage 1: H = gelu(Xg @ ew1), f-major
            ht = persist.tile([128, KR, cap], BF16)       # H^T [1024, cap]
            if ncl < cap:
                nc.vector.memset(ht[:, :, ncl:cap], 0.0)
            w1e = wstream.tile([128, KD, HR], BF16, tag="w", name="ew1p")
            nc.sync.dma_start(out=w1e[:, :, :], in_=ew1[:, :, :])
            w2e = wstream.tile([128, KR, D], BF16, tag="w", name="ew2p")
            nc.sync.dma_start(out=w2e[:, :, :], in_=ew2[:, :, :])
            for m in range(KR):
                cb = 0
                for bs in blks:
                    pa = psA.tile([128, bs], F32, tag="pa")
                    for k in range(KD):
                        nc.tensor.matmul(pa[:, :],
                                         w1e[:, k, m * 128:(m + 1) * 128],
                                         xgq[:, k, cb:cb + bs],
                                         start=(k == 0), stop=(k == KD - 1))
                    nc.scalar.activation(ht[:, m, cb:cb + bs], pa[:, :], AF.Gelu)
                    cb += bs

            # ---- routed expert stage 2: out_rt = gate * (H @ ew2), token-major
            for c in range(nchr):
                ot = opool.tile([128, D], BF16, tag="ot")
                for dh in range(2):
                    py = psY.tile([128, 512], F32, tag="py")
                    for k in range(KR):
                        nc.tensor.matmul(py[:, :],
                                         ht[:, k, c * 128:(c + 1) * 128],
                                         w2e[:, k, dh * 512:(dh + 1) * 512],
                                         start=(k == 0), stop=(k == KR - 1))
                    nc.vector.tensor_copy(ot[:, dh * 512:(dh + 1) * 512],
                                          py[:, :])
                nc.sync.dma_start(out=out_rt[:, c, :], in_=ot[:, :])
    nc.compile()
    return nc


def _get_nc(cap=None, ncl=None):
    if cap is None:
        cap = _CACHE.get("cap", 1152)
    if ncl is None:
        ncl = _CACHE.get("ncl", cap)
    if _CACHE.get("cap") != cap or _CACHE.get("ncl") != ncl or "nc" not in _CACHE:
        _CACHE["nc"] = _build(cap, ncl)
        _CACHE["cap"] = cap
        _CACHE["ncl"] = ncl
    return _CACHE["nc"]


def _route(inputs):
    """Host router: scores, top-2 indices, gates (matches reference numerics
    well inside the ~4e-5 min score gap of the selection)."""
    xf = np.ascontiguousarray(inputs["x"], dtype=np.float32).reshape(B * T, D)
    rw = np.asarray(inputs["router_w"], dtype=np.float64)
    bias = np.asarray(inputs["router_bias"], dtype=np.float64)
    logits = xf.astype(np.float64) @ rw.T                  # [N, E]
    s = 1.0 / (1.0 + np.exp(-logits))
    sel = s + bias
    n = sel.shape[0]
    ar = np.arange(n)
    i0 = np.argmax(sel, axis=1)
    sel2 = sel.copy()
    sel2[ar, i0] = -np.inf
    i1 = np.argmax(sel2, axis=1)
    s0 = s[ar, i0].astype(np.float32)
    s1 = s[ar, i1].astype(np.float32)
    den = s0 + s1
    g0 = np.where(den > 1e-9, s0 / (den + 1e-9), np.float32(0.5)).astype(np.float32)
    g1 = np.where(den > 1e-9, s1 / (den + 1e-9), np.float32(0.5)).astype(np.float32)
    return xf, i0, i1, g0, g1


def _dispatch(xf, i0, i1, g0, g1):
    """Build per-expert gathered token buffers, padded to a uniform capacity,
    pre-packed to the kernel's [128, KD, cap] bf16 tile layout."""
    idxs, gates = [], []
    for e in range(E):
        m0 = np.nonzero(i0 == e)[0]
        m1 = np.nonzero(i1 == e)[0]
        idx = np.concatenate([m0, m1])
        gv = np.concatenate([g0[m0], g1[m1]])
        idxs.append(idx)
        gates.append(gv)
    maxc = max(len(ix) for ix in idxs)
    cap = max(((maxc + 127) // 128) * 128, 256)
    _dispatch.maxc = maxc
    xgs, gvs = [], []
    _dispatch.gvs = gvs
    xfb = xf.astype(NPBF16)
    for e in range(E):
        cnt = len(idxs[e])
        xg = np.zeros((D, cap), dtype=NPBF16)
        xg[:, :cnt] = xfb[idxs[e]].T
        # [D, cap] -> [kp=128, kt=8, cap]
        xg = np.ascontiguousarray(xg.reshape(KD, 128, cap).transpose(1, 0, 2))
        xgs.append(xg)
        gvs.append(np.asarray(gates[e], dtype=np.float32))
    return cap, idxs, xgs, gvs


def _pack_shared(inputs):
    sw1 = np.asarray(inputs["sw1"], dtype=np.float32).astype(NPBF16)
    sw3 = np.asarray(inputs["sw3"], dtype=np.float32).astype(NPBF16)
    sw2 = np.asarray(inputs["sw2"], dtype=np.float32).astype(NPBF16)
    # [D, HS] x2 -> [h2=16, kp=128, 2, kt=8, 128]
    sw13 = np.stack([sw1.reshape(KD, 128, KS, 128),
                     sw3.reshape(KD, 128, KS, 128)])    # [2, kt, kp, h2, 128]
    sw13p = np.ascontiguousarray(sw13.transpose(3, 2, 0, 1, 4))
    # [HS, D] -> [dh=2, kp=128, kk=16, 512]
    sw2p = np.ascontiguousarray(
        sw2.reshape(KS, 128, 2, 512).transpose(2, 1, 0, 3))
    return sw13p, sw2p


def _pack_expert(w):
    # [1024, 1024] -> [kp=128, kt=8, 1024]
    wb = np.asarray(w, dtype=np.float32).astype(NPBF16)
    return np.ascontiguousarray(wb.reshape(KD, 128, wb.shape[1]).transpose(1, 0, 2))


def _make_in_maps(inputs):
    xf, i0, i1, g0, g1 = _route(inputs)
    cap, idxs, xgs, gvs = _dispatch(xf, i0, i1, g0, g1)
    maxc = _dispatch.maxc
    sw13p, sw2p = _pack_shared(inputs)
    ew1 = np.asarray(inputs["ew1"])
    ew2 = np.asarray(inputs["ew2"])
    xfb = xf.astype(NPBF16)
    in_maps = []
    for c in range(N_CORES):
        xsl = xfb[c * NTOK:(c + 1) * NTOK]                # [512, 1024] bf16
        xtp = np.ascontiguousarray(
            xsl.T.reshape(KD, 128, NTOK).transpose(1, 0, 2))  # [128, 8, 512]
        in_maps.append({
            "xt": xtp,
            "xg": xgs[c],
            "sw13": sw13p, "sw2": sw2p,
            "ew1": _pack_expert(ew1[c]),
            "ew2": _pack_expert(ew2[c]),
        })
    return in_maps, cap, idxs, _dispatch.gvs, maxc


def kernel(x, router_w, router_bias, sw1, sw3, sw2, ew1, ew2):
    inputs = dict(x=x, router_w=router_w, router_bias=router_bias,
                  sw1=sw1, sw3=sw3, sw2=sw2, ew1=ew1, ew2=ew2)
    in_maps, cap, idxs, gvs, maxc = _make_in_maps(inputs)
    nc = _get_nc(cap, maxc)
    res = run_bass_kernel_spmd(nc, in_maps, core_ids=list(range(N_CORES)))
    out = np.empty((B * T, D), dtype=np.float32)
    for c in range(N_CORES):
        # [128, NCH, D] -> [NTOK, D]
        osh = np.asarray(res.results[c]["out_sh"]).astype(np.float32).transpose(1, 0, 2).reshape(NTOK, D)
        out[c * NTOK:(c + 1) * NTOK] = osh
    for e in range(E):
        cnt = len(idxs[e])
        ort = np.asarray(res.results[e]["out_rt"]).astype(np.float32).transpose(1, 0, 2).reshape(cap, D)
        out[idxs[e]] += gvs[e][:, None] * ort[:cnt]
    return out.reshape(B, T, D)
